# revision 19
# baseline (speedup 1.0000x reference)
"""Binary-weight 3x3 conv (sign(W)), NCHW, stride 1, pad 1, on 8 trn2 cores.

Full inputs:  x [32,128,56,56] f32, W [256,128,3,3] f32
Full output:  out [32,256,56,56] f32

Strategy: data-parallel over batch (4 images/core). Per core, 1D Winograd
F(2,3) along H: output rows are produced in pairs from 4 transform-domain
terms, cutting PE work 1.5x vs direct conv (12 matmuls of N=392 per
14-row block instead of 18 matmul-units).

 - input transform (HOST, fp32 -> bf16): per row-pair tile t:
     x~0 = xp[2t] - xp[2t+2], x~1 = xp[2t+1] + xp[2t+2],
     x~2 = xp[2t+2] - xp[2t+1], x~3 = xp[2t+1] - xp[2t+3]
 - weight transform (host, exact in bf16): per kw:
     w~ = [w0, (w0+w1+w2)/2, (w0-w1+w2)/2, w2]  with w = sign(W) in {+-1}
 - PE: m_i = sum_kw w~_i[kw]^T @ x~_i[.., kw:kw+56], PSUM-accumulated
   over kw (4 banks per block, 8 banks double-buffered)
 - inverse transform, spread so no engine exceeds ~70% of PE time:
     ACT:  c0 = bf16(m0), c3 = bf16(m3)        (1-input PSUM drains)
     DVE:  s = m1+m2, d = m1-m2 (PSUM, bf16 out); oe = s+c0 (bf16)
     Pool: oo = d-c3 (bf16)
   oe/oo land in DRAM rows 2t / 2t+1 via strided output-DMA patterns.

Output is bf16 (halves DMA) and upcast on host. Warmup matmuls on a
zeroed tile ramp the PE p-state during the input DMA.
"""

import numpy as np
import ml_dtypes

import concourse.bacc as bacc
import concourse.mybir as mybir
from concourse.tile import TileContext
from concourse.bass_utils import run_bass_kernel_spmd

N_CORES = 8
IMGS = 4          # images per core (32 / 8)
C = 128           # input channels = contraction dim = partitions
O = 256           # output channels
H = WD = 56
HP = WP = 58      # padded spatial
P = 128
N_WARM = 8

RT = 7            # row-pair tiles per block
NBLK = 4          # blocks per (img, half): 4 * RT * 2 = 56 rows
NT = 28           # row-pair tiles per image

BF16 = mybir.dt.bfloat16
F32 = mybir.dt.float32


def build_nc():
    nc = bacc.Bacc(None, target_bir_lowering=False)
    # host-transformed input, 4 Winograd terms per image
    x = nc.dram_tensor("x", [IMGS, C, 4, NT, WP], BF16, kind="ExternalInput")
    wb = nc.dram_tensor("wb", [C, 2, 4, 3, P], BF16, kind="ExternalInput")
    out = nc.dram_tensor("out", [IMGS, O, H, WD], BF16, kind="ExternalOutput")

    with TileContext(nc) as tc:
        with (
            tc.tile_pool(name="wpool", bufs=1) as wpool,
            tc.tile_pool(name="xpool", bufs=1) as xpool,
            tc.tile_pool(name="cpool", bufs=6) as cpool,
            tc.tile_pool(name="opool", bufs=5) as opool,
            tc.tile_pool(name="psum", bufs=8, space="PSUM") as psum_pool,
        ):
            wt = wpool.tile([P, 2, 4, 3, P], BF16, name="wt")
            wsc = wpool.tile([P, 512], BF16, name="wsc")
            nc.vector.memset(wsc[:], 0.0)

            # transform-domain tiles: [P, IMGS, term, tile-halves]
            xw = xpool.tile([P, IMGS, 4, NT, WP], BF16, name="xw")

            # DMA dispatch order = urgency order. Sync-queue dispatch costs
            # ~600ns per DMA, so keep the count low: moderately-grained img0
            # chunks (first matmul gates on one term's tiles + weights),
            # then whole images
            for i in (1, 2, 0, 3):
                nc.sync.dma_start(out=xw[:, 0, i, 0:14], in_=x[0, :, i, 0:14])
                nc.sync.dma_start(out=wt[:, 0, i], in_=wb[:, 0, i])
            for i in (1, 2, 0, 3):
                nc.sync.dma_start(out=xw[:, 0, i, 14:28], in_=x[0, :, i, 14:28])
            nc.sync.dma_start(out=wt[:, 1], in_=wb[:, 1])
            for img in range(1, IMGS):
                for i in (1, 2, 0, 3):
                    nc.sync.dma_start(out=xw[:, img, i], in_=x[img, :, i])

            # warmup: PE activity during the input DMA (p-state ramp)
            warm = psum_pool.tile([P, RT, WD], F32, name="warm", tag="pst")
            for _ in range(N_WARM):
                nc.tensor.matmul(
                    warm[:], lhsT=wsc[:, :P], rhs=wsc[:, :RT * WD],
                    start=True, stop=True,
                )

            for img in range(IMGS):
                for half in range(2):
                    for blk in range(NBLK):
                        t0 = RT * blk
                        pst = [
                            psum_pool.tile([P, RT, WD], F32,
                                           name=f"m{i}", tag="pst")
                            for i in range(4)
                        ]
                        for i in (1, 2, 3, 0):
                            for kw in range(3):
                                nc.tensor.matmul(
                                    pst[i][:],
                                    lhsT=wt[:, half, i, kw, :],
                                    rhs=xw[:, img, i, t0:t0 + RT, kw:kw + WD],
                                    start=(kw == 0),
                                    stop=(kw == 2),
                                )

                        s = cpool.tile([P, RT, WD], BF16, name="s", tag="s")
                        d = cpool.tile([P, RT, WD], BF16, name="d", tag="d")
                        c0 = cpool.tile([P, RT, WD], BF16, name="c0", tag="c0")
                        c1 = cpool.tile([P, RT, WD], F32, name="c1", tag="c1")
                        c3 = cpool.tile([P, RT, WD], BF16, name="c3", tag="c3")
                        # one ot tile + one output DMA per pair of blocks
                        # (last img-half: per-block DMAs for a shorter tail)
                        last_ih = (img == IMGS - 1 and half == 1)
                        if blk % 2 == 0 or last_ih:
                            otp = opool.tile([P, 4 * RT, WD], BF16,
                                             name="ot", tag="ot")
                        ot = otp[:, (blk % 2) * 2 * RT:
                                 (blk % 2) * 2 * RT + 2 * RT]

                        # DVE may read at most one PSUM operand per op, so
                        # m1 goes through an ACT copy first; remaining work
                        # alternates between DVE and Pool to balance queues
                        nc.scalar.copy(out=c1[:], in_=pst[1][:])
                        nc.vector.tensor_add(s[:], c1[:], pst[2][:])
                        nc.vector.tensor_sub(d[:], c1[:], pst[2][:])
                        nc.scalar.copy(out=c3[:], in_=pst[3][:])
                        if blk % 2 == 0:
                            nc.vector.tensor_copy(c0[:], pst[0][:])
                            nc.vector.tensor_add(
                                ot[:, 0:2 * RT:2], s[:], c0[:])
                            nc.gpsimd.tensor_sub(
                                ot[:, 1:2 * RT:2], d[:], c3[:])
                        else:
                            nc.scalar.copy(out=c0[:], in_=pst[0][:])
                            nc.gpsimd.tensor_add(
                                ot[:, 0:2 * RT:2], s[:], c0[:])
                            nc.gpsimd.tensor_sub(
                                ot[:, 1:2 * RT:2], d[:], c3[:])

                        if last_ih:
                            nc.sync.dma_start(
                                out=out[img, half * P:(half + 1) * P,
                                        2 * t0:2 * t0 + 2 * RT, :],
                                in_=ot[:],
                            )
                        elif blk % 2 == 1:
                            nc.sync.dma_start(
                                out=out[img, half * P:(half + 1) * P,
                                        2 * t0 - 2 * RT:2 * t0 + 2 * RT, :],
                                in_=otp[:],
                            )
    nc.compile()
    return nc


_NC_CACHE = None


def _get_nc():
    global _NC_CACHE
    if _NC_CACHE is None:
        _NC_CACHE = build_nc()
    return _NC_CACHE


def prep_inputs(x: np.ndarray, W: np.ndarray):
    """Host prep: pad + Winograd input/weight transforms, shard over cores."""
    x = np.asarray(x, dtype=np.float32)
    n = x.shape[0]
    xp = np.zeros((n, C, HP, WP), dtype=np.float32)
    xp[:, :, 1:H + 1, 1:WD + 1] = x
    ev = xp[:, :, 0:2 * NT:2]      # rows 0,2,..,54
    od = xp[:, :, 1:2 * NT:2]      # rows 1,3,..,55
    e2 = xp[:, :, 2:2 * NT + 1:2]  # rows 2,4,..,56
    o3 = xp[:, :, 3:2 * NT + 2:2]  # rows 3,5,..,57
    xwt = np.stack(
        [ev - e2, od + e2, e2 - od, od - o3], axis=2
    ).astype(ml_dtypes.bfloat16)   # [n, C, 4, 28, 58]
    xs = xwt.reshape(N_CORES, IMGS, C, 4, NT, WP)

    # G = F(2,3) weight transform along kh; entries are exact in bf16
    G = np.array(
        [[1, 0, 0], [0.5, 0.5, 0.5], [0.5, -0.5, 0.5], [0, 0, 1]],
        dtype=np.float32,
    )
    wsign = np.sign(np.asarray(W)).astype(np.float32)  # [O,C,3,3]
    wtr = np.einsum("ih,ochw->ociw", G, wsign)         # [O,C,4,3]
    wbt = np.ascontiguousarray(
        wtr.reshape(2, P, C, 4, 3).transpose(2, 0, 3, 4, 1)
    ).astype(ml_dtypes.bfloat16)                       # [C,2,4,3,128]
    return [
        {"x": np.ascontiguousarray(xs[c]), "wb": wbt}
        for c in range(N_CORES)
    ]


def kernel(x: np.ndarray, W: np.ndarray) -> np.ndarray:
    nc = _get_nc()
    in_maps = prep_inputs(x, W)
    res = run_bass_kernel_spmd(nc, in_maps, core_ids=list(range(N_CORES)))
    outs = [res.results[c]["out"] for c in range(N_CORES)]
    return np.concatenate(outs, axis=0).astype(np.float32)


# revision 20
# speedup vs baseline: 1.0387x; 1.0387x over previous
"""Binary-weight 3x3 conv (sign(W)), NCHW, stride 1, pad 1, on 8 trn2 cores.

Full inputs:  x [32,128,56,56] f32, W [256,128,3,3] f32
Full output:  out [32,256,56,56] f32

Strategy: data-parallel over batch (4 images/core). Per core, 1D Winograd
F(2,3) along H: output rows are produced in pairs from 4 transform-domain
terms, cutting PE work 1.5x vs direct conv (12 matmuls of N=392 per
14-row block instead of 18 matmul-units).

 - input transform (HOST, fp32 -> bf16): per row-pair tile t:
     x~0 = xp[2t] - xp[2t+2], x~1 = xp[2t+1] + xp[2t+2],
     x~2 = xp[2t+2] - xp[2t+1], x~3 = xp[2t+1] - xp[2t+3]
 - weight transform (host, exact in bf16): per kw:
     w~ = [w0, (w0+w1+w2)/2, (w0-w1+w2)/2, w2]  with w = sign(W) in {+-1}
 - PE: m_i = sum_kw w~_i[kw]^T @ x~_i[.., kw:kw+56], PSUM-accumulated
   over kw (4 banks per block, 8 banks double-buffered)
 - inverse transform, spread so no engine exceeds ~70% of PE time:
     ACT:  c0 = bf16(m0), c3 = bf16(m3)        (1-input PSUM drains)
     DVE:  s = m1+m2, d = m1-m2 (PSUM, bf16 out); oe = s+c0 (bf16)
     Pool: oo = d-c3 (bf16)
   oe/oo land in DRAM rows 2t / 2t+1 via strided output-DMA patterns.

Output is bf16 (halves DMA) and upcast on host. Warmup matmuls on a
zeroed tile ramp the PE p-state during the input DMA.
"""

import numpy as np
import ml_dtypes

import concourse.bacc as bacc
import concourse.mybir as mybir
from concourse.tile import TileContext
from concourse.bass_utils import run_bass_kernel_spmd

N_CORES = 8
IMGS = 4          # images per core (32 / 8)
C = 128           # input channels = contraction dim = partitions
O = 256           # output channels
H = WD = 56
HP = WP = 58      # padded spatial
P = 128
N_WARM = 8

RT = 7            # row-pair tiles per block
NBLK = 4          # blocks per (img, half): 4 * RT * 2 = 56 rows
NT = 28           # row-pair tiles per image

BF16 = mybir.dt.bfloat16
F32 = mybir.dt.float32


def build_nc():
    nc = bacc.Bacc(None, target_bir_lowering=False)
    # host-transformed input, 4 Winograd terms per image
    x = nc.dram_tensor("x", [IMGS, C, 4, NT, WP], BF16, kind="ExternalInput")
    wb = nc.dram_tensor("wb", [C, 2, 4, 3, P], BF16, kind="ExternalInput")
    out = nc.dram_tensor("out", [IMGS, O, H, WD], BF16, kind="ExternalOutput")

    with TileContext(nc) as tc:
        with (
            tc.tile_pool(name="wpool", bufs=1) as wpool,
            tc.tile_pool(name="xpool", bufs=1) as xpool,
            tc.tile_pool(name="cpool", bufs=4) as cpool,
            tc.tile_pool(name="opool", bufs=4) as opool,
            tc.tile_pool(name="psum", bufs=8, space="PSUM") as psum_pool,
        ):
            wt = wpool.tile([P, 2, 4, 3, P], BF16, name="wt")
            wsc = wpool.tile([P, 512], BF16, name="wsc")
            nc.vector.memset(wsc[:], 0.0)

            # transform-domain tiles: [P, IMGS, term, tile-halves]
            xw = xpool.tile([P, IMGS, 4, NT, WP], BF16, name="xw")

            # DMA dispatch order = urgency order. Sync-queue dispatch costs
            # ~600ns per DMA, so keep the count low: moderately-grained img0
            # chunks (first matmul gates on one term's tiles + weights),
            # then whole images
            for i in (1, 2, 0, 3):
                nc.sync.dma_start(out=xw[:, 0, i, 0:14], in_=x[0, :, i, 0:14])
                nc.sync.dma_start(out=wt[:, 0, i], in_=wb[:, 0, i])
            for i in (1, 2, 0, 3):
                nc.sync.dma_start(out=xw[:, 0, i, 14:28], in_=x[0, :, i, 14:28])
            nc.sync.dma_start(out=wt[:, 1], in_=wb[:, 1])
            for img in range(1, IMGS):
                for i in (1, 2, 0, 3):
                    nc.sync.dma_start(out=xw[:, img, i], in_=x[img, :, i])

            # warmup: PE activity during the input DMA (p-state ramp)
            warm = psum_pool.tile([P, RT, WD], F32, name="warm", tag="pst")
            for _ in range(N_WARM):
                nc.tensor.matmul(
                    warm[:], lhsT=wsc[:, :P], rhs=wsc[:, :RT * WD],
                    start=True, stop=True,
                )

            for img in range(IMGS):
                for half in range(2):
                    for blk in range(NBLK):
                        t0 = RT * blk
                        pst = [
                            psum_pool.tile([P, RT, WD], F32,
                                           name=f"m{i}", tag="pst")
                            for i in range(4)
                        ]
                        for i in (1, 2, 0, 3):
                            for kw in range(3):
                                nc.tensor.matmul(
                                    pst[i][:],
                                    lhsT=wt[:, half, i, kw, :],
                                    rhs=xw[:, img, i, t0:t0 + RT, kw:kw + WD],
                                    start=(kw == 0),
                                    stop=(kw == 2),
                                )

                        s = cpool.tile([P, RT, WD], BF16, name="s", tag="s")
                        d = cpool.tile([P, RT, WD], BF16, name="d", tag="d")
                        c0 = cpool.tile([P, RT, WD], BF16, name="c0", tag="c0")
                        c1 = cpool.tile([P, RT, WD], F32, name="c1", tag="c1")
                        c3 = cpool.tile([P, RT, WD], BF16, name="c3", tag="c3")
                        # one ot tile + one output DMA per pair of blocks
                        # (last img-half: per-block DMAs for a shorter tail)
                        last_ih = (img == IMGS - 1 and half == 1)
                        if blk % 2 == 0 or last_ih:
                            otp = opool.tile([P, 4 * RT, WD], BF16,
                                             name="ot", tag="ot")
                        ot = otp[:, (blk % 2) * 2 * RT:
                                 (blk % 2) * 2 * RT + 2 * RT]

                        # DVE may read at most one PSUM operand per op, so
                        # m1 goes through an ACT copy first; remaining work
                        # alternates between DVE and Pool to balance queues
                        nc.scalar.copy(out=c1[:], in_=pst[1][:])
                        nc.vector.tensor_add(s[:], c1[:], pst[2][:])
                        nc.vector.tensor_sub(d[:], c1[:], pst[2][:])
                        nc.scalar.copy(out=c3[:], in_=pst[3][:])
                        if blk % 2 == 0:
                            nc.vector.tensor_copy(c0[:], pst[0][:])
                            nc.vector.tensor_add(
                                ot[:, 0:2 * RT:2], s[:], c0[:])
                            nc.gpsimd.tensor_sub(
                                ot[:, 1:2 * RT:2], d[:], c3[:])
                        else:
                            nc.scalar.copy(out=c0[:], in_=pst[0][:])
                            nc.gpsimd.tensor_add(
                                ot[:, 0:2 * RT:2], s[:], c0[:])
                            nc.gpsimd.tensor_sub(
                                ot[:, 1:2 * RT:2], d[:], c3[:])

                        if last_ih:
                            nc.sync.dma_start(
                                out=out[img, half * P:(half + 1) * P,
                                        2 * t0:2 * t0 + 2 * RT, :],
                                in_=ot[:],
                            )
                        elif blk % 2 == 1:
                            nc.sync.dma_start(
                                out=out[img, half * P:(half + 1) * P,
                                        2 * t0 - 2 * RT:2 * t0 + 2 * RT, :],
                                in_=otp[:],
                            )
    nc.compile()
    return nc


_NC_CACHE = None


def _get_nc():
    global _NC_CACHE
    if _NC_CACHE is None:
        _NC_CACHE = build_nc()
    return _NC_CACHE


def prep_inputs(x: np.ndarray, W: np.ndarray):
    """Host prep: pad + Winograd input/weight transforms, shard over cores."""
    x = np.asarray(x, dtype=np.float32)
    n = x.shape[0]
    xp = np.zeros((n, C, HP, WP), dtype=np.float32)
    xp[:, :, 1:H + 1, 1:WD + 1] = x
    ev = xp[:, :, 0:2 * NT:2]      # rows 0,2,..,54
    od = xp[:, :, 1:2 * NT:2]      # rows 1,3,..,55
    e2 = xp[:, :, 2:2 * NT + 1:2]  # rows 2,4,..,56
    o3 = xp[:, :, 3:2 * NT + 2:2]  # rows 3,5,..,57
    xwt = np.stack(
        [ev - e2, od + e2, e2 - od, od - o3], axis=2
    ).astype(ml_dtypes.bfloat16)   # [n, C, 4, 28, 58]
    xs = xwt.reshape(N_CORES, IMGS, C, 4, NT, WP)

    # G = F(2,3) weight transform along kh; entries are exact in bf16
    G = np.array(
        [[1, 0, 0], [0.5, 0.5, 0.5], [0.5, -0.5, 0.5], [0, 0, 1]],
        dtype=np.float32,
    )
    wsign = np.sign(np.asarray(W)).astype(np.float32)  # [O,C,3,3]
    wtr = np.einsum("ih,ochw->ociw", G, wsign)         # [O,C,4,3]
    wbt = np.ascontiguousarray(
        wtr.reshape(2, P, C, 4, 3).transpose(2, 0, 3, 4, 1)
    ).astype(ml_dtypes.bfloat16)                       # [C,2,4,3,128]
    return [
        {"x": np.ascontiguousarray(xs[c]), "wb": wbt}
        for c in range(N_CORES)
    ]


def kernel(x: np.ndarray, W: np.ndarray) -> np.ndarray:
    nc = _get_nc()
    in_maps = prep_inputs(x, W)
    res = run_bass_kernel_spmd(nc, in_maps, core_ids=list(range(N_CORES)))
    outs = [res.results[c]["out"] for c in range(N_CORES)]
    return np.concatenate(outs, axis=0).astype(np.float32)
